# revision 13
# baseline (speedup 1.0000x reference)
"""Trainium2 Bass kernel for a top-2 MoE layer (B=2, T=2048, D=1024, F=4096, E=8).

Strategy (expert-parallel, per sharding hint):
  Launch 1 (router, data-parallel over tokens): each of 8 cores computes
    logits = x_slice @ Wr (f32r on the PE, x-stationary so logits land
    directly as [tok, E]), then top-2 combine weights via the sigmoid
    identity  p1/(p1+p2) = sigmoid(l1-l2)  (the bottom-6 softmax terms
    cancel).  Output: comb[512, 8] fp32 per core.
  Host dispatch (data movement only): tokens are gathered per expert
    (all-to-all performed by the host), padded to a static capacity.
  Launch 2 (expert FFN, expert-parallel): core e holds expert e's W1/W2
    fully resident in SBUF (bf16: 128 KiB/partition), computes
    y = c * (gelu(x@W1+b1)@W2 + b2) for its gathered tokens in a single
    pass over F per token chunk; y is written once, in bf16.
  Launch 3 (combine): out[t] = yA[t] + yB[t] — the two selected experts'
    scaled outputs per token (bf16), added on-device into fp32.

All arithmetic is on-device; the host only reshapes/gathers/concats.
"""

import numpy as np

import concourse.bacc as bacc
import concourse.mybir as mybir
import concourse.tile as tile
from concourse import bass_utils
from concourse.tile_rust import add_dep_helper

F32 = mybir.dt.float32
F32R = mybir.dt.float32r
BF16 = mybir.dt.bfloat16
NPBF16 = mybir.dt.np(mybir.dt.bfloat16)
AX = mybir.AxisListType
ALU = mybir.AluOpType
ACT_F = mybir.ActivationFunctionType

B, T, D, F, E = 2, 2048, 1024, 4096, 8
NTOK = B * T              # 4096
NCORES = 8
TOK_PER_CORE = NTOK // NCORES  # 512
DO = D // 128             # 8 d-tiles
FT = F // 128             # 32 f-tiles

_cache = {}


def _run(nc, in_maps, trace=False, **kw):
    return bass_utils.run_bass_kernel_spmd(
        nc, in_maps, core_ids=list(range(NCORES)), trace=trace, **kw
    )


# ----------------------------------------------------------------- router ---
def build_router():
    """Per core: xT [128, 4, 8, 128] f32r (xT[p,tt,do,c] = x[tt*128+c, do*128+p]),
    Wr [128, 8, 8] f32r -> comb [128, 4, 8] fp32 (comb[p,tt,e] for token
    tt*128+p)."""
    if "router" in _cache:
        return _cache["router"]
    TT = TOK_PER_CORE // 128  # 4 token tiles
    nc = bacc.Bacc("TRN2", target_bir_lowering=False, debug=False)
    # NOTE: plain fp32 (not f32r) — ~0.1% of tokens have a 2nd-vs-3rd logit
    # gap under 2e-4, and reduced-precision logits can flip their second
    # expert, which costs ~0.7 token-norms of error each.
    xT_d = nc.dram_tensor("xT_sl", [128, TT * DO * 128], F32,
                          kind="ExternalInput").ap()
    wr_d = nc.dram_tensor("Wr", [128, DO * E], F32, kind="ExternalInput").ap()
    out_d = nc.dram_tensor("comb", [128, TT * E], F32, kind="ExternalOutput").ap()
    xT_ap = xT_d.rearrange("p (t o c) -> p t o c", t=TT, o=DO)

    with tile.TileContext(nc) as tc:
        with (
            tc.tile_pool(name="pool", bufs=1) as pool,
            tc.tile_pool(name="work", bufs=2) as work,
            tc.tile_pool(name="psum", bufs=2, space="PSUM") as psum,
            tc.tile_pool(name="psw", bufs=1, space="PSUM") as psw,
        ):
            xT_sb = pool.tile([128, TT, DO, 128], F32)
            wr_sb = pool.tile([128, DO, E], F32)
            comb_sb = pool.tile([128, TT, E], F32)
            warm_sb = pool.tile([128, 256], BF16)
            dum = pool.tile([1, 1], F32)
            dum2 = pool.tile([1, 1], F32)

            # Sigmoid table preload + PE p-state ramp during the x DMA.
            nc.gpsimd.memset(dum[:], 0.0)
            nc.scalar.activation(dum2[:], dum[:], ACT_F.Sigmoid)
            nc.gpsimd.memset(warm_sb[:], 0.0)
            warm_ps = psw.tile([128, 256], F32)
            for _ in range(12):
                nc.tensor.matmul(warm_ps[:], warm_sb[:, :128], warm_sb[:],
                                 start=True, stop=True)

            nc.scalar.dma_start(wr_sb[:], wr_d.rearrange("p (o e) -> p o e", o=DO))
            xt_engines = [nc.sync, nc.gpsimd, nc.sync, nc.gpsimd]
            for tt in range(TT):
                xt_engines[tt].dma_start(xT_sb[:, tt], xT_ap[:, tt])

            for tt in range(TT):
                lp = psum.tile([128, E], F32, tag="lp")
                for do in range(DO):
                    nc.tensor.matmul(
                        lp[:], xT_sb[:, tt, do, :], wr_sb[:, do, :],
                        start=(do == 0), stop=(do == DO - 1),
                    )
                l = work.tile([128, E], F32, tag="l")
                nc.vector.tensor_copy(l[:], lp[:])
                mx1 = work.tile([128, 1], F32, tag="mx1")
                nc.vector.reduce_max(mx1[:], l[:], axis=AX.X)
                eq1 = work.tile([128, E], F32, tag="eq1")
                nc.vector.tensor_scalar(eq1[:], l[:], mx1[:], None, op0=ALU.is_equal)
                lm = work.tile([128, E], F32, tag="lm")
                nc.vector.scalar_tensor_tensor(
                    lm[:], eq1[:], -1e30, l[:], op0=ALU.mult, op1=ALU.add
                )
                mx2 = work.tile([128, 1], F32, tag="mx2")
                nc.vector.reduce_max(mx2[:], lm[:], axis=AX.X)
                dlt = work.tile([128, 1], F32, tag="dlt")
                nc.vector.scalar_tensor_tensor(
                    dlt[:], mx1[:], -1.0, mx2[:], op0=ALU.mult, op1=ALU.add
                )
                s2 = work.tile([128, 1], F32, tag="s2")
                nc.scalar.activation(s2[:], dlt[:], ACT_F.Sigmoid)
                s1m2 = work.tile([128, 1], F32, tag="s1m2")
                nc.vector.tensor_scalar(s1m2[:], s2[:], -2.0, 1.0,
                                        op0=ALU.mult, op1=ALU.add)
                ge = work.tile([128, E], F32, tag="ge")
                nc.vector.tensor_scalar(ge[:], l[:], mx2[:], None, op0=ALU.is_ge)
                t1 = work.tile([128, E], F32, tag="t1")
                nc.vector.tensor_scalar_mul(t1[:], ge[:], s2[:])
                nc.vector.scalar_tensor_tensor(
                    comb_sb[:, tt, :], eq1[:], s1m2[:], t1[:],
                    op0=ALU.mult, op1=ALU.add,
                )

            nc.scalar.dma_start(out_d.rearrange("p (t e) -> p t e", t=TT),
                                comb_sb[:])
    nc.compile()
    _cache["router"] = nc
    return nc


# -------------------------------------------------------------------- ffn ---
def build_ffn(cap, with_b1=False, with_b2=False):
    """Per core (expert e), all bf16 except biases/cvec:
    xTg [128, DO*cap], W1e [128, FT*DO*128], W2e [128, FT*D],
    b1e [128, FT] f32, b2e/ones bf16, cvec [128, TT] f32
    -> y [128, TT*D] bf16 with y = cvec * (gelu(xg@W1 + b1) @ W2 + b2).

    W1+W2 are fully SBUF-resident (128 KiB/partition in bf16); F is
    processed in a single pass per token chunk, so y is written once."""
    key = ("ffn", cap, with_b1, with_b2)
    if key in _cache:
        return _cache[key]
    assert cap % 32 == 0
    TTILES = -(-cap // 128)
    chunks = _chunk_split(cap)
    CHUNKMAX = max(cs for _, cs in chunks)

    nc = bacc.Bacc("TRN2", target_bir_lowering=False, debug=False)
    xT_d = nc.dram_tensor("xTg", [128, DO * cap], BF16, kind="ExternalInput").ap()
    w1_d = nc.dram_tensor("W1e", [128, FT * DO * 128], BF16,
                          kind="ExternalInput").ap()
    w2_d = nc.dram_tensor("W2e", [128, FT * D], BF16, kind="ExternalInput").ap()
    b1_d = nc.dram_tensor("b1e", [128, FT], F32, kind="ExternalInput").ap()
    b2_d = nc.dram_tensor("b2e", [1, D], BF16, kind="ExternalInput").ap()
    ones_d = nc.dram_tensor("ones", [1, 128], BF16, kind="ExternalInput").ap()
    cv_d = nc.dram_tensor("cvec", [128, TTILES], F32, kind="ExternalInput").ap()
    y_d = nc.dram_tensor("y", [128, TTILES * D], BF16, kind="ExternalOutput").ap()
    w1_ap = w1_d.rearrange("p (f o c) -> p f o c", f=FT, o=DO)
    w2_ap = w2_d.rearrange("p (f d) -> p f d", f=FT)
    y_ap = y_d.rearrange("p (t d) -> p t d", t=TTILES)

    with tile.TileContext(nc) as tc:
        with (
            tc.tile_pool(name="res", bufs=1) as res,
            tc.tile_pool(name="xtp", bufs=2) as xtp,
            tc.tile_pool(name="htp", bufs=1) as htp,
            tc.tile_pool(name="ysp", bufs=3) as ysp,
            tc.tile_pool(name="ps1", bufs=3, space="PSUM") as ps1,
            tc.tile_pool(name="ps2", bufs=2, space="PSUM") as ps2,
        ):
            w1_sb = res.tile([128, FT, DO, 128], BF16)   # 64 KiB/partition
            w2_sb = res.tile([128, FT, D], BF16)         # 64 KiB/partition
            cv_sb = res.tile([128, TTILES], F32)
            b1_sb = res.tile([128, FT], F32)
            b2_sb = res.tile([1, D], BF16)
            ones_sb = res.tile([1, 128], BF16)
            warm_sb = res.tile([128, 512], BF16)
            gdum = res.tile([1, 1], BF16)

            # Gelu table preload + PE ramp while the first weight DMAs land.
            nc.gpsimd.memset(warm_sb[:], 0.0)
            nc.scalar.activation(gdum[:], warm_sb[:1, :1], ACT_F.Gelu)
            for _ in range(26):
                warm_ps = ps1.tile([128, CHUNKMAX], F32, tag="hp")
                nc.tensor.matmul(warm_ps[:], warm_sb[:, :128],
                                 warm_sb[:, :CHUNKMAX], start=True, stop=True)

            nc.scalar.dma_start(cv_sb[:], cv_d[:])
            nc.scalar.dma_start(b1_sb[:], b1_d[:])
            nc.scalar.dma_start(b2_sb[:], b2_d[:])
            nc.scalar.dma_start(ones_sb[:], ones_d[:])

            # W1 in f-major slabs (first matmul needs only slab 0).  The DMA
            # hardware round-robins packets across ALL outstanding transfers,
            # so W2 must not be in flight while stage 1 waits on W1: the W2
            # dma_starts get an artificial dependency on an early chunk-0
            # gelu (installed below) so they issue only once W1's critical
            # head has landed.  xT chunks ride the scalar queue.
            for lo, hi, eng in [(0, 2, nc.sync), (2, 8, nc.gpsimd),
                                (8, 20, nc.sync), (20, 32, nc.gpsimd)]:
                eng.dma_start(w1_sb[:, lo:hi], w1_ap[:, lo:hi])
            w2_dmas = [
                eng.dma_start(w2_sb[:, lo:hi], w2_ap[:, lo:hi])
                for lo, hi, eng in [(0, 8, nc.gpsimd), (8, 20, nc.sync),
                                    (20, 32, nc.gpsimd)]
            ]

            def load_xt(c0, cs):
                xT_sb = xtp.tile([128, DO, CHUNKMAX], BF16, tag="xt")
                off = DO * c0
                nc.scalar.dma_start(
                    xT_sb[:, :, :cs],
                    xT_d[:, off:off + DO * cs].rearrange("p (o t) -> p o t", o=DO),
                )
                return xT_sb

            xt_next = load_xt(*chunks[0])
            for ci, (c0, cs) in enumerate(chunks):
                xT_sb = xt_next
                if ci + 1 < len(chunks):
                    xt_next = load_xt(*chunks[ci + 1])
                hT_sb = htp.tile([128, FT, CHUNKMAX], BF16, tag="ht")
                # stage 1: hT[f, tok] = gelu(W1.T @ xT + b1)
                for ft in range(FT):
                    hp = ps1.tile([128, CHUNKMAX], F32, tag="hp")
                    for do in range(DO):
                        nc.tensor.matmul(
                            hp[:, :cs], w1_sb[:, ft, do, :], xT_sb[:, do, :cs],
                            start=(do == 0), stop=(do == DO - 1),
                        )
                    if with_b1:
                        g = nc.scalar.activation(hT_sb[:, ft, :cs], hp[:, :cs],
                                                 ACT_F.Gelu,
                                                 bias=b1_sb[:, ft:ft + 1])
                    else:
                        g = nc.scalar.activation(hT_sb[:, ft, :cs], hp[:, :cs],
                                                 ACT_F.Gelu)
                    if ci == 0 and ft == 2:
                        for wd in w2_dmas:
                            add_dep_helper(g.ins, wd.ins,
                                           reason="W2 prefetch after W1 head")
                # stage 2: y[tok, d] = cvec * (hT.T @ W2 (+ b2))
                for tt in range(-(-cs // 128)):
                    gt = c0 // 128 + tt
                    m = min(128, cs - tt * 128)
                    yp = ps2.tile([128, D], F32, tag="yp")
                    for fo in range(FT):
                        for n in range(D // 512):
                            nc.tensor.matmul(
                                yp[:m, n * 512:(n + 1) * 512],
                                hT_sb[:, fo, tt * 128:tt * 128 + m],
                                w2_sb[:, fo, n * 512:(n + 1) * 512],
                                start=(fo == 0),
                                stop=(fo == FT - 1 and not with_b2),
                            )
                    if with_b2:
                        for n in range(D // 512):
                            nc.tensor.matmul(
                                yp[:m, n * 512:(n + 1) * 512],
                                ones_sb[:, :m], b2_sb[:, n * 512:(n + 1) * 512],
                                start=False, stop=True,
                            )
                    y_sb = ysp.tile([128, D], BF16, tag="y")
                    nc.vector.tensor_scalar_mul(y_sb[:m, :], yp[:m, :],
                                                cv_sb[:m, gt:gt + 1])
                    nc.sync.dma_start(y_ap[:m, gt, :], y_sb[:m, :])
    nc.compile()
    _cache[key] = nc
    return nc


# ---------------------------------------------------------------- combine ---
def build_combine():
    """Per core: packed a, b [128, (T/128)*D] bf16 -> o = a + b fp32.

    Host packs A[t, d] -> Ah[p, tt*D + d] with t = tt*128 + p so every DMA is
    one contiguous segment per partition."""
    if "comb" in _cache:
        return _cache["comb"]
    W = (TOK_PER_CORE // 128) * D  # 4096
    NP = 8  # pieces
    PW = W // NP
    nc = bacc.Bacc("TRN2", target_bir_lowering=False, debug=False)
    a_d = nc.dram_tensor("a", [128, W], BF16, kind="ExternalInput").ap()
    b_d = nc.dram_tensor("b", [128, W], BF16, kind="ExternalInput").ap()
    o_d = nc.dram_tensor("o", [128, W], BF16, kind="ExternalOutput").ap()
    with tile.TileContext(nc) as tc:
        with tc.tile_pool(name="pool", bufs=4) as pool:
            for pc in range(NP):
                sl = slice(pc * PW, (pc + 1) * PW)
                at = pool.tile([128, PW], BF16, tag="a")
                bt = pool.tile([128, PW], BF16, tag="b")
                ot = pool.tile([128, PW], BF16, tag="o")
                nc.sync.dma_start(at[:], a_d[:, sl])
                nc.gpsimd.dma_start(bt[:], b_d[:, sl])
                nc.vector.tensor_add(ot[:], at[:], bt[:])
                nc.scalar.dma_start(o_d[:, sl], ot[:])
    nc.compile()
    _cache["comb"] = nc
    return nc


# ----------------------------------------------------------------- driver ---
def _chunk_split(cap):
    """Split cap (multiple of 32) into chunks: all 128-aligned starts, sizes
    multiples of 128 except the last (multiple of 32), each >=256 and <=512."""
    full = cap // 128
    rem = cap % 128
    k = -(-cap // 512)
    counts = [full // k + (1 if i < full % k else 0) for i in range(k)]
    chunks, c0 = [], 0
    for i, n in enumerate(counts):
        cs = n * 128 + (rem if i == k - 1 else 0)
        chunks.append((c0, cs))
        c0 += cs
    return chunks


def _moe_forward(x2d, Wr, W1, b1, W2, b2, trace=False):
    """x2d: [NTOK, D] fp32. Returns (out [NTOK, D] fp32, exec_ns_total|None)."""
    TT = TOK_PER_CORE // 128

    # --- launch 1: router ---
    rnc = build_router()
    wrh = np.ascontiguousarray(
        Wr.reshape(DO, 128, E).transpose(1, 0, 2).reshape(128, -1))
    in_maps = [
        {"xT_sl": np.ascontiguousarray(
            x2d[c * TOK_PER_CORE:(c + 1) * TOK_PER_CORE]
            .reshape(TT, 128, DO, 128).transpose(3, 0, 2, 1).reshape(128, -1)),
         "Wr": wrh}
        for c in range(NCORES)
    ]
    rres = _run(rnc, in_maps, trace=trace)
    comb = np.concatenate(
        [rres.results[c]["comb"].reshape(128, TT, E)
         .transpose(1, 0, 2).reshape(TOK_PER_CORE, E) for c in range(NCORES)],
        axis=0)
    exec_ns = rres.exec_time_ns or 0
    per_launch = [rres.exec_time_ns]

    # --- host dispatch (data movement only) ---
    top2 = np.argpartition(-comb, 1, axis=1)[:, :2]  # [NTOK, 2]
    sel_lists, cvals = [], []
    for e in range(E):
        sel = np.nonzero((top2 == e).any(axis=1))[0]
        sel_lists.append(sel)
        cvals.append(comb[sel, e])
    counts = np.array([len(s) for s in sel_lists])
    MAXCAP = 3072
    nbatch = max(1, -(-int(counts.max()) // MAXCAP))
    cap = int(max(256, -(-(-(-counts.max() // nbatch)) // 32) * 32))

    fnc = build_ffn(cap, with_b1=bool(np.any(b1)), with_b2=bool(np.any(b2)))
    chunks = _chunk_split(cap)
    ttiles = -(-cap // 128)
    ones_in = np.ones((1, 128), NPBF16)
    x2d_bf = x2d.astype(NPBF16)
    w_packed = [
        {"W1e": np.ascontiguousarray(
            W1[e].reshape(DO, 128, FT, 128).transpose(1, 2, 0, 3)
            .reshape(128, -1).astype(NPBF16)),
         "b1e": np.ascontiguousarray(b1[e].reshape(FT, 128).T),
         "W2e": np.ascontiguousarray(
            W2[e].reshape(FT, 128, D).transpose(1, 0, 2)
            .reshape(128, -1).astype(NPBF16)),
         "b2e": np.ascontiguousarray(b2[e].astype(NPBF16)).reshape(1, D)}
        for e in range(E)
    ]
    ys = [np.zeros((0, D), NPBF16) for _ in range(E)]
    for bi in range(nbatch):
        in_maps = []
        for e in range(E):
            sel_b = sel_lists[e][bi * cap:(bi + 1) * cap]
            cv_b = cvals[e][bi * cap:(bi + 1) * cap]
            n_e = len(sel_b)
            xsel = np.zeros((cap, D), NPBF16)
            xsel[:n_e] = x2d_bf[sel_b]
            xg = np.concatenate(
                [xsel[c0:c0 + cs].reshape(cs, DO, 128).transpose(2, 1, 0)
                 .reshape(128, -1) for (c0, cs) in chunks], axis=1)
            cv = np.zeros(ttiles * 128, np.float32)
            cv[:n_e] = cv_b
            cv = np.ascontiguousarray(cv.reshape(ttiles, 128).T)
            in_maps.append({"xTg": np.ascontiguousarray(xg), "ones": ones_in,
                            "cvec": cv, **w_packed[e]})
        fres = _run(fnc, in_maps, trace=trace)
        ys = [np.concatenate([
            ys[e],
            fres.results[e]["y"].reshape(128, ttiles, D)
            .transpose(1, 0, 2).reshape(ttiles * 128, D)[:cap]])
            for e in range(E)]
        exec_ns += fres.exec_time_ns or 0
        per_launch.append(fres.exec_time_ns)

    # --- host: build per-token (A, B) contribution rows (gather only) ---
    slot = np.zeros((NTOK, E), np.int64)
    for e in range(E):
        slot[sel_lists[e], e] = np.arange(counts[e])
    e1, e2v = top2[:, 0], top2[:, 1]
    A = np.empty((NTOK, D), NPBF16)
    Bm = np.empty((NTOK, D), NPBF16)
    for e in range(E):
        m1 = e1 == e
        A[m1] = ys[e][slot[m1, e]]
        m2 = e2v == e
        Bm[m2] = ys[e][slot[m2, e]]

    # --- launch 3: combine ---
    cnc = build_combine()

    def pack(m, c):
        sl = m[c * TOK_PER_CORE:(c + 1) * TOK_PER_CORE]
        return np.ascontiguousarray(
            sl.reshape(TOK_PER_CORE // 128, 128, D).transpose(1, 0, 2)
            .reshape(128, -1))

    in_maps = [{"a": pack(A, c), "b": pack(Bm, c)} for c in range(NCORES)]
    cres = _run(cnc, in_maps, trace=trace)
    out = np.concatenate(
        [cres.results[c]["o"].astype(np.float32)
         .reshape(128, TOK_PER_CORE // 128, D)
         .transpose(1, 0, 2).reshape(TOK_PER_CORE, D) for c in range(NCORES)],
        axis=0)
    exec_ns += cres.exec_time_ns or 0
    per_launch.append(cres.exec_time_ns)
    if trace:
        print(f"per-launch exec ns (router, ffn, combine): {per_launch}")
        _moe_forward.last = (rres, fres, cres)
    return out, (exec_ns if trace else None)


def kernel(x, Wr, W1, b1, W2, b2):
    x = np.asarray(x, np.float32)
    out, _ = _moe_forward(
        x.reshape(NTOK, D),
        np.asarray(Wr, np.float32),
        np.asarray(W1, np.float32),
        np.asarray(b1, np.float32),
        np.asarray(W2, np.float32),
        np.asarray(b2, np.float32),
        trace=False,
    )
    return out.reshape(B, T, D)


# revision 22
# speedup vs baseline: 1.1183x; 1.1183x over previous
"""Trainium2 Bass kernel for a top-2 MoE layer (B=2, T=2048, D=1024, F=4096, E=8).

Strategy (expert-parallel, per sharding hint):
  Launch 1 (router, data-parallel over tokens): each of 8 cores computes
    logits = x_slice @ Wr (f32r on the PE, x-stationary so logits land
    directly as [tok, E]), then top-2 combine weights via the sigmoid
    identity  p1/(p1+p2) = sigmoid(l1-l2)  (the bottom-6 softmax terms
    cancel).  Output: comb[512, 8] fp32 per core.
  Host dispatch (data movement only): tokens are gathered per expert
    (all-to-all performed by the host), padded to a static capacity.
  Launch 2 (expert FFN, expert-parallel): core e holds expert e's W1/W2
    fully resident in SBUF (bf16: 128 KiB/partition), computes
    y = c * (gelu(x@W1+b1)@W2 + b2) for its gathered tokens in a single
    pass over F per token chunk; y is written once, in bf16.
  Launch 3 (combine): out[t] = yA[t] + yB[t] — the two selected experts'
    scaled outputs per token (bf16), added on-device into fp32.

All arithmetic is on-device; the host only reshapes/gathers/concats.
"""

import numpy as np

import concourse.bacc as bacc
import concourse.mybir as mybir
import concourse.tile as tile
from concourse import bass_utils

F32 = mybir.dt.float32
F32R = mybir.dt.float32r
BF16 = mybir.dt.bfloat16
NPBF16 = mybir.dt.np(mybir.dt.bfloat16)
AX = mybir.AxisListType
ALU = mybir.AluOpType
ACT_F = mybir.ActivationFunctionType

B, T, D, F, E = 2, 2048, 1024, 4096, 8
NTOK = B * T              # 4096
NCORES = 8
TOK_PER_CORE = NTOK // NCORES  # 512
DO = D // 128             # 8 d-tiles
FT = F // 128             # 32 f-tiles

_cache = {}


def _run(nc, in_maps, trace=False, **kw):
    return bass_utils.run_bass_kernel_spmd(
        nc, in_maps, core_ids=list(range(NCORES)), trace=trace, **kw
    )


# ----------------------------------------------------------------- router ---
def build_router():
    """Per core: xT [128, 4, 8, 128] f32r (xT[p,tt,do,c] = x[tt*128+c, do*128+p]),
    Wr [128, 8, 8] f32r -> comb [128, 4, 8] fp32 (comb[p,tt,e] for token
    tt*128+p)."""
    if "router" in _cache:
        return _cache["router"]
    TT = TOK_PER_CORE // 128  # 4 token tiles
    nc = bacc.Bacc("TRN2", target_bir_lowering=False, debug=False)
    # NOTE: plain fp32 (not f32r) — ~0.1% of tokens have a 2nd-vs-3rd logit
    # gap under 2e-4, and reduced-precision logits can flip their second
    # expert, which costs ~0.7 token-norms of error each.
    xT_d = nc.dram_tensor("xT_sl", [128, TT * DO * 128], F32,
                          kind="ExternalInput").ap()
    wr_d = nc.dram_tensor("Wr", [128, DO * E], F32, kind="ExternalInput").ap()
    out_d = nc.dram_tensor("comb", [128, TT * E], F32, kind="ExternalOutput").ap()
    xT_ap = xT_d.rearrange("p (t o c) -> p t o c", t=TT, o=DO)

    with tile.TileContext(nc) as tc:
        with (
            tc.tile_pool(name="pool", bufs=1) as pool,
            tc.tile_pool(name="work", bufs=2) as work,
            tc.tile_pool(name="psum", bufs=2, space="PSUM") as psum,
            tc.tile_pool(name="psw", bufs=1, space="PSUM") as psw,
        ):
            xT_sb = pool.tile([128, TT, DO, 128], F32)
            wr_sb = pool.tile([128, DO, E], F32)
            comb_sb = pool.tile([128, TT, E], F32)
            warm_sb = pool.tile([128, 256], BF16)
            dum = pool.tile([1, 1], F32)
            dum2 = pool.tile([1, 1], F32)

            # Sigmoid table preload + PE p-state ramp during the x DMA.
            nc.gpsimd.memset(dum[:], 0.0)
            nc.scalar.activation(dum2[:], dum[:], ACT_F.Sigmoid)
            nc.gpsimd.memset(warm_sb[:], 0.0)
            warm_ps = psw.tile([128, 256], F32)
            for _ in range(16):
                nc.tensor.matmul(warm_ps[:], warm_sb[:, :128], warm_sb[:],
                                 start=True, stop=True)

            # tt0 split across the three DMA queues so the first token tile
            # lands ASAP; later tiles are queued behind it (per-queue FIFO
            # gives consumption-order priority).
            nc.scalar.dma_start(wr_sb[:], wr_d.rearrange("p (o e) -> p o e", o=DO))
            qs = [nc.sync, nc.gpsimd, nc.scalar]
            for i, (lo, hi) in enumerate([(0, 3), (3, 6), (6, 8)]):
                qs[i].dma_start(xT_sb[:, 0, lo:hi], xT_ap[:, 0, lo:hi])
            for tt in range(1, TT):
                qs[tt - 1].dma_start(xT_sb[:, tt], xT_ap[:, tt])

            for tt in range(TT):
                lp = psum.tile([128, E], F32, tag="lp")
                for do in range(DO):
                    nc.tensor.matmul(
                        lp[:], xT_sb[:, tt, do, :], wr_sb[:, do, :],
                        start=(do == 0), stop=(do == DO - 1),
                    )
                l = work.tile([128, E], F32, tag="l")
                nc.vector.tensor_copy(l[:], lp[:])
                mx1 = work.tile([128, 1], F32, tag="mx1")
                nc.vector.reduce_max(mx1[:], l[:], axis=AX.X)
                eq1 = work.tile([128, E], F32, tag="eq1")
                nc.vector.tensor_scalar(eq1[:], l[:], mx1[:], None, op0=ALU.is_equal)
                lm = work.tile([128, E], F32, tag="lm")
                nc.vector.scalar_tensor_tensor(
                    lm[:], eq1[:], -1e30, l[:], op0=ALU.mult, op1=ALU.add
                )
                mx2 = work.tile([128, 1], F32, tag="mx2")
                nc.vector.reduce_max(mx2[:], lm[:], axis=AX.X)
                dlt = work.tile([128, 1], F32, tag="dlt")
                nc.vector.scalar_tensor_tensor(
                    dlt[:], mx1[:], -1.0, mx2[:], op0=ALU.mult, op1=ALU.add
                )
                s2 = work.tile([128, 1], F32, tag="s2")
                nc.scalar.activation(s2[:], dlt[:], ACT_F.Sigmoid)
                s1m2 = work.tile([128, 1], F32, tag="s1m2")
                nc.vector.tensor_scalar(s1m2[:], s2[:], -2.0, 1.0,
                                        op0=ALU.mult, op1=ALU.add)
                ge = work.tile([128, E], F32, tag="ge")
                nc.vector.tensor_scalar(ge[:], l[:], mx2[:], None, op0=ALU.is_ge)
                t1 = work.tile([128, E], F32, tag="t1")
                nc.vector.tensor_scalar_mul(t1[:], ge[:], s2[:])
                nc.vector.scalar_tensor_tensor(
                    comb_sb[:, tt, :], eq1[:], s1m2[:], t1[:],
                    op0=ALU.mult, op1=ALU.add,
                )

            nc.scalar.dma_start(out_d.rearrange("p (t e) -> p t e", t=TT),
                                comb_sb[:])
    nc.compile()
    _cache["router"] = nc
    return nc


# -------------------------------------------------------------------- ffn ---
def build_ffn(cap, with_b1=False, with_b2=False):
    """Per core (expert e), all bf16 except biases/cvec:
    xTg [128, DO*cap], W1e [128, FT*DO*128], W2e [128, FT*D],
    b1e [128, FT] f32, b2e/ones bf16, cvec [128, TT] f32
    -> y [128, TT*D] bf16 with y = cvec * (gelu(xg@W1 + b1) @ W2 + b2).

    W1+W2 are fully SBUF-resident (128 KiB/partition in bf16); F is
    processed in a single pass per token chunk, so y is written once."""
    key = ("ffn", cap, with_b1, with_b2)
    if key in _cache:
        return _cache[key]
    assert cap % 32 == 0
    TTILES = -(-cap // 128)
    chunks = _chunk_split(cap)
    CHUNKMAX = max(cs for _, cs in chunks)

    nc = bacc.Bacc("TRN2", target_bir_lowering=False, debug=False)
    xT_d = nc.dram_tensor("xTg", [128, DO * cap], BF16, kind="ExternalInput").ap()
    w1_d = nc.dram_tensor("W1e", [128, FT * DO * 128], BF16,
                          kind="ExternalInput").ap()
    w2_d = nc.dram_tensor("W2e", [128, FT * D], BF16, kind="ExternalInput").ap()
    b1_d = nc.dram_tensor("b1e", [128, FT], F32, kind="ExternalInput").ap()
    b2_d = nc.dram_tensor("b2e", [1, D], BF16, kind="ExternalInput").ap()
    ones_d = nc.dram_tensor("ones", [1, 128], BF16, kind="ExternalInput").ap()
    cv_d = nc.dram_tensor("cvec", [128, TTILES], F32, kind="ExternalInput").ap()
    y_d = nc.dram_tensor("y", [128, TTILES * D], BF16, kind="ExternalOutput").ap()
    w1_ap = w1_d.rearrange("p (f o c) -> p f o c", f=FT, o=DO)
    w2_ap = w2_d.rearrange("p (f d) -> p f d", f=FT)
    y_ap = y_d.rearrange("p (t d) -> p t d", t=TTILES)

    with tile.TileContext(nc) as tc:
        with (
            tc.tile_pool(name="res", bufs=1) as res,
            tc.tile_pool(name="xtp", bufs=2) as xtp,
            tc.tile_pool(name="htp", bufs=1) as htp,
            tc.tile_pool(name="ysp", bufs=3) as ysp,
            tc.tile_pool(name="ps1", bufs=3, space="PSUM") as ps1,
            tc.tile_pool(name="ps2", bufs=2, space="PSUM") as ps2,
        ):
            w1_sb = res.tile([128, FT, DO, 128], BF16)   # 64 KiB/partition
            w2_sb = res.tile([128, FT, D], BF16)         # 64 KiB/partition
            cv_sb = res.tile([128, TTILES], F32)
            b1_sb = res.tile([128, FT], F32)
            b2_sb = res.tile([1, D], BF16)
            ones_sb = res.tile([1, 128], BF16)
            warm_sb = res.tile([128, 512], BF16)
            gdum = res.tile([1, 1], BF16)

            # Gelu table preload + PE ramp while the first weight DMAs land.
            nc.gpsimd.memset(warm_sb[:], 0.0)
            nc.scalar.activation(gdum[:], warm_sb[:1, :1], ACT_F.Gelu)
            for _ in range(26):
                warm_ps = ps1.tile([128, CHUNKMAX], F32, tag="hp")
                nc.tensor.matmul(warm_ps[:], warm_sb[:, :128],
                                 warm_sb[:, :CHUNKMAX], start=True, stop=True)

            nc.gpsimd.dma_start(cv_sb[:], cv_d[:])
            nc.gpsimd.dma_start(b1_sb[:], b1_d[:])
            nc.gpsimd.dma_start(b2_sb[:], b2_d[:])
            nc.gpsimd.dma_start(ones_sb[:], ones_d[:])

            # The DMA hardware round-robins packets fairly across ALL
            # outstanding transfers, but a single engine queue is processed
            # in order and alone reaches full HBM bandwidth.  So ALL weight
            # transfers go on the sync queue, in exactly the order the PE
            # consumes them (W1 f-slabs, then W2 fo-slabs) — priority by
            # ordering.  xT chunks ride the scalar queue; aux + y-out use
            # gpsimd.
            for lo, hi in [(0, 2), (2, 4), (4, 8), (8, 12), (12, 16),
                           (16, 20), (20, 24), (24, 28), (28, 32)]:
                nc.sync.dma_start(w1_sb[:, lo:hi], w1_ap[:, lo:hi])
            for lo, hi in [(0, 8), (8, 16), (16, 24), (24, 32)]:
                nc.sync.dma_start(w2_sb[:, lo:hi], w2_ap[:, lo:hi])

            def load_xt(c0, cs):
                xT_sb = xtp.tile([128, DO, CHUNKMAX], BF16, tag="xt")
                off = DO * c0
                nc.scalar.dma_start(
                    xT_sb[:, :, :cs],
                    xT_d[:, off:off + DO * cs].rearrange("p (o t) -> p o t", o=DO),
                )
                return xT_sb

            xt_next = load_xt(*chunks[0])
            for ci, (c0, cs) in enumerate(chunks):
                xT_sb = xt_next
                if ci + 1 < len(chunks):
                    xt_next = load_xt(*chunks[ci + 1])
                hT_sb = htp.tile([128, FT, CHUNKMAX], BF16, tag="ht")
                # stage 1: hT[f, tok] = gelu(W1.T @ xT + b1)
                for ft in range(FT):
                    hp = ps1.tile([128, CHUNKMAX], F32, tag="hp")
                    for do in range(DO):
                        nc.tensor.matmul(
                            hp[:, :cs], w1_sb[:, ft, do, :], xT_sb[:, do, :cs],
                            start=(do == 0), stop=(do == DO - 1),
                        )
                    if with_b1:
                        nc.scalar.activation(hT_sb[:, ft, :cs], hp[:, :cs],
                                             ACT_F.Gelu, bias=b1_sb[:, ft:ft + 1])
                    else:
                        nc.scalar.activation(hT_sb[:, ft, :cs], hp[:, :cs],
                                             ACT_F.Gelu)
                # stage 2: y[tok, d] = cvec * (hT.T @ W2 (+ b2))
                for tt in range(-(-cs // 128)):
                    gt = c0 // 128 + tt
                    m = min(128, cs - tt * 128)
                    yp = ps2.tile([128, D], F32, tag="yp")
                    for fo in range(FT):
                        for n in range(D // 512):
                            nc.tensor.matmul(
                                yp[:m, n * 512:(n + 1) * 512],
                                hT_sb[:, fo, tt * 128:tt * 128 + m],
                                w2_sb[:, fo, n * 512:(n + 1) * 512],
                                start=(fo == 0),
                                stop=(fo == FT - 1 and not with_b2),
                            )
                    if with_b2:
                        for n in range(D // 512):
                            nc.tensor.matmul(
                                yp[:m, n * 512:(n + 1) * 512],
                                ones_sb[:, :m], b2_sb[:, n * 512:(n + 1) * 512],
                                start=False, stop=True,
                            )
                    y_sb = ysp.tile([128, D], BF16, tag="y")
                    nc.vector.tensor_scalar_mul(y_sb[:m, :], yp[:m, :],
                                                cv_sb[:m, gt:gt + 1])
                    nc.gpsimd.dma_start(y_ap[:m, gt, :], y_sb[:m, :])
    nc.compile()
    _cache[key] = nc
    return nc


# ---------------------------------------------------------------- combine ---
def build_combine():
    """Per core: packed a, b [128, (T/128)*D] bf16 -> o = a + b fp32.

    Host packs A[t, d] -> Ah[p, tt*D + d] with t = tt*128 + p so every DMA is
    one contiguous segment per partition."""
    if "comb" in _cache:
        return _cache["comb"]
    W = (TOK_PER_CORE // 128) * D  # 4096
    NP = 8  # pieces
    PW = W // NP
    nc = bacc.Bacc("TRN2", target_bir_lowering=False, debug=False)
    a_d = nc.dram_tensor("a", [128, W], BF16, kind="ExternalInput").ap()
    b_d = nc.dram_tensor("b", [128, W], BF16, kind="ExternalInput").ap()
    o_d = nc.dram_tensor("o", [128, W], BF16, kind="ExternalOutput").ap()
    with tile.TileContext(nc) as tc:
        with tc.tile_pool(name="pool", bufs=4) as pool:
            for pc in range(NP):
                sl = slice(pc * PW, (pc + 1) * PW)
                at = pool.tile([128, PW], BF16, tag="a")
                bt = pool.tile([128, PW], BF16, tag="b")
                ot = pool.tile([128, PW], BF16, tag="o")
                nc.sync.dma_start(at[:], a_d[:, sl])
                nc.gpsimd.dma_start(bt[:], b_d[:, sl])
                nc.vector.tensor_add(ot[:], at[:], bt[:])
                nc.scalar.dma_start(o_d[:, sl], ot[:])
    nc.compile()
    _cache["comb"] = nc
    return nc


# ----------------------------------------------------------------- driver ---
def _chunk_split(cap):
    """Split cap (multiple of 32) into chunks: all 128-aligned starts, sizes
    multiples of 128 except the last (multiple of 32), each >=256 and <=512."""
    full = cap // 128
    rem = cap % 128
    k = -(-cap // 512)
    counts = [full // k + (1 if i < full % k else 0) for i in range(k)]
    chunks, c0 = [], 0
    for i, n in enumerate(counts):
        cs = n * 128 + (rem if i == k - 1 else 0)
        chunks.append((c0, cs))
        c0 += cs
    return chunks


def _moe_forward(x2d, Wr, W1, b1, W2, b2, trace=False):
    """x2d: [NTOK, D] fp32. Returns (out [NTOK, D] fp32, exec_ns_total|None)."""
    TT = TOK_PER_CORE // 128

    # --- launch 1: router ---
    rnc = build_router()
    wrh = np.ascontiguousarray(
        Wr.reshape(DO, 128, E).transpose(1, 0, 2).reshape(128, -1))
    in_maps = [
        {"xT_sl": np.ascontiguousarray(
            x2d[c * TOK_PER_CORE:(c + 1) * TOK_PER_CORE]
            .reshape(TT, 128, DO, 128).transpose(3, 0, 2, 1).reshape(128, -1)),
         "Wr": wrh}
        for c in range(NCORES)
    ]
    rres = _run(rnc, in_maps, trace=trace)
    comb = np.concatenate(
        [rres.results[c]["comb"].reshape(128, TT, E)
         .transpose(1, 0, 2).reshape(TOK_PER_CORE, E) for c in range(NCORES)],
        axis=0)
    exec_ns = rres.exec_time_ns or 0
    per_launch = [rres.exec_time_ns]

    # --- host dispatch (data movement only) ---
    top2 = np.argpartition(-comb, 1, axis=1)[:, :2]  # [NTOK, 2]
    sel_lists, cvals = [], []
    for e in range(E):
        sel = np.nonzero((top2 == e).any(axis=1))[0]
        sel_lists.append(sel)
        cvals.append(comb[sel, e])
    counts = np.array([len(s) for s in sel_lists])
    MAXCAP = 3072
    nbatch = max(1, -(-int(counts.max()) // MAXCAP))
    cap = int(max(256, -(-(-(-counts.max() // nbatch)) // 32) * 32))

    fnc = build_ffn(cap, with_b1=bool(np.any(b1)), with_b2=bool(np.any(b2)))
    chunks = _chunk_split(cap)
    ttiles = -(-cap // 128)
    ones_in = np.ones((1, 128), NPBF16)
    x2d_bf = x2d.astype(NPBF16)
    w_packed = [
        {"W1e": np.ascontiguousarray(
            W1[e].reshape(DO, 128, FT, 128).transpose(1, 2, 0, 3)
            .reshape(128, -1).astype(NPBF16)),
         "b1e": np.ascontiguousarray(b1[e].reshape(FT, 128).T),
         "W2e": np.ascontiguousarray(
            W2[e].reshape(FT, 128, D).transpose(1, 0, 2)
            .reshape(128, -1).astype(NPBF16)),
         "b2e": np.ascontiguousarray(b2[e].astype(NPBF16)).reshape(1, D)}
        for e in range(E)
    ]
    ys = [np.zeros((0, D), NPBF16) for _ in range(E)]
    for bi in range(nbatch):
        in_maps = []
        for e in range(E):
            sel_b = sel_lists[e][bi * cap:(bi + 1) * cap]
            cv_b = cvals[e][bi * cap:(bi + 1) * cap]
            n_e = len(sel_b)
            xsel = np.zeros((cap, D), NPBF16)
            xsel[:n_e] = x2d_bf[sel_b]
            xg = np.concatenate(
                [xsel[c0:c0 + cs].reshape(cs, DO, 128).transpose(2, 1, 0)
                 .reshape(128, -1) for (c0, cs) in chunks], axis=1)
            cv = np.zeros(ttiles * 128, np.float32)
            cv[:n_e] = cv_b
            cv = np.ascontiguousarray(cv.reshape(ttiles, 128).T)
            in_maps.append({"xTg": np.ascontiguousarray(xg), "ones": ones_in,
                            "cvec": cv, **w_packed[e]})
        fres = _run(fnc, in_maps, trace=trace)
        ys = [np.concatenate([
            ys[e],
            fres.results[e]["y"].reshape(128, ttiles, D)
            .transpose(1, 0, 2).reshape(ttiles * 128, D)[:cap]])
            for e in range(E)]
        exec_ns += fres.exec_time_ns or 0
        per_launch.append(fres.exec_time_ns)

    # --- host: build per-token (A, B) contribution rows (gather only) ---
    slot = np.zeros((NTOK, E), np.int64)
    for e in range(E):
        slot[sel_lists[e], e] = np.arange(counts[e])
    e1, e2v = top2[:, 0], top2[:, 1]
    A = np.empty((NTOK, D), NPBF16)
    Bm = np.empty((NTOK, D), NPBF16)
    for e in range(E):
        m1 = e1 == e
        A[m1] = ys[e][slot[m1, e]]
        m2 = e2v == e
        Bm[m2] = ys[e][slot[m2, e]]

    # --- launch 3: combine ---
    cnc = build_combine()

    def pack(m, c):
        sl = m[c * TOK_PER_CORE:(c + 1) * TOK_PER_CORE]
        return np.ascontiguousarray(
            sl.reshape(TOK_PER_CORE // 128, 128, D).transpose(1, 0, 2)
            .reshape(128, -1))

    in_maps = [{"a": pack(A, c), "b": pack(Bm, c)} for c in range(NCORES)]
    cres = _run(cnc, in_maps, trace=trace)
    out = np.concatenate(
        [cres.results[c]["o"].astype(np.float32)
         .reshape(128, TOK_PER_CORE // 128, D)
         .transpose(1, 0, 2).reshape(TOK_PER_CORE, D) for c in range(NCORES)],
        axis=0)
    exec_ns += cres.exec_time_ns or 0
    per_launch.append(cres.exec_time_ns)
    if trace:
        print(f"per-launch exec ns (router, ffn, combine): {per_launch}")
        _moe_forward.last = (rres, fres, cres)
    return out, (exec_ns if trace else None)


def kernel(x, Wr, W1, b1, W2, b2):
    x = np.asarray(x, np.float32)
    out, _ = _moe_forward(
        x.reshape(NTOK, D),
        np.asarray(Wr, np.float32),
        np.asarray(W1, np.float32),
        np.asarray(b1, np.float32),
        np.asarray(W2, np.float32),
        np.asarray(b2, np.float32),
        trace=False,
    )
    return out.reshape(B, T, D)
